# revision 1
# baseline (speedup 1.0000x reference)
"""Fused transformer block (LN -> MHA -> LN -> FFN) on 8 TRN2 NeuronCores.

Sharding: core c handles batch (c // 2), token half (c % 2).  The host rolls
each batch's tokens so every core's "own" tokens are rows 0..T-1 of its x
input; K/V are computed for all S tokens locally (duplicated within the
pair), so the 8 cores are fully independent (no collectives).

Numerics: LayerNorm affine + all linear biases are folded into the weights
on the host; matmuls run in bf16 with fp32 PSUM accumulation; softmax skips
max-subtraction (|scores| <= ~3 for LN'd inputs) and gets its denominator
from a ones-column appended to V.

Layout strategy: scores are computed transposed (scoresT[s,q] = kT.T @ qT)
so the exp'd attention matrix feeds the ctx matmul as the stationary
operand directly -- the big S*S transpose never happens.  Projections that
need per-outdim bias fold it into the PSUM->SBUF copy (transposed outputs:
per-partition scalar; normal outputs: broadcast row tile).

SBUF pools are LIFO per (space, side); long-lived attention tensors live on
the "left" stack, phase-transient ones on the "right" stack.
"""

from contextlib import ExitStack

import ml_dtypes
import numpy as np

import concourse.bass as bass
import concourse.mybir as mybir
import concourse.tile as tile
from concourse import bacc
from concourse.masks import make_identity

F32 = mybir.dt.float32
BF16 = mybir.dt.bfloat16
AF = mybir.ActivationFunctionType
ALU = mybir.AluOpType

B_FULL = 4
S_FULL = 2048
D_FULL = 1024
H_FULL = 16
FF_FULL = 2048
HD = 64
EPS = 1e-5
N_CORES = 8

LAST_EXEC_NS = None
LAST_RESULTS = None
LAST_NC = None


def build_nc(S=S_FULL, T=S_FULL // 2, D=D_FULL, H=H_FULL, FF=FF_FULL,
             gelu_af=AF.Gelu):
    """Build the single-core (SPMD) Bass program.

    S: total tokens per batch (K/V length), T: own tokens (Q length),
    D: model dim, H: heads (H*64 == D), FF: hidden dim.
    """
    assert H * HD == D
    P = 128
    DT = D // P           # d-tiles (contraction tiles over D)
    TT_ALL = S // P       # token tiles over full sequence
    TT_OWN = T // P       # token tiles over own tokens
    FT = FF // P          # ff tiles
    QC = min(512, T)      # q chunk (columns per scores matmul)
    NQC = T // QC
    QSUB = QC // P        # q subtiles per chunk
    NC_D = min(512, D)    # matmul N chunk over D
    DCH = D // NC_D
    HPD = P // HD         # heads per 128-partition tile (=2)
    GS = min(512, D)      # bn_stats group size
    NG = D // GS

    nc = bacc.Bacc("TRN2", target_bir_lowering=False, debug=False,
                   enable_asserts=False, num_devices=N_CORES)

    x_d = nc.dram_tensor("x", [S, D], F32, kind="ExternalInput").ap()
    xb_d = nc.dram_tensor("xb", [S, D], BF16, kind="ExternalInput").ap()
    wq_d = nc.dram_tensor("wq", [D, D], BF16, kind="ExternalInput").ap()
    wk_d = nc.dram_tensor("wk", [D, D], BF16, kind="ExternalInput").ap()
    wv_d = nc.dram_tensor("wv", [D, D], BF16, kind="ExternalInput").ap()
    wo_d = nc.dram_tensor("wo", [D, D], BF16, kind="ExternalInput").ap()
    w1_d = nc.dram_tensor("w1", [D, FF], BF16, kind="ExternalInput").ap()
    w2_d = nc.dram_tensor("w2", [FF, D], BF16, kind="ExternalInput").ap()
    bq_d = nc.dram_tensor("bq", [D], F32, kind="ExternalInput").ap()
    bk_d = nc.dram_tensor("bk", [D], F32, kind="ExternalInput").ap()
    bv_d = nc.dram_tensor("bv", [D], F32, kind="ExternalInput").ap()
    bo_d = nc.dram_tensor("bo", [D], F32, kind="ExternalInput").ap()
    b1_d = nc.dram_tensor("b1", [FF], F32, kind="ExternalInput").ap()
    b2_d = nc.dram_tensor("b2", [D], F32, kind="ExternalInput").ap()
    out_d = nc.dram_tensor("out", [T, D], F32, kind="ExternalOutput").ap()

    def bcast(ap_1d, n):
        return bass.AP(tensor=ap_1d.tensor, offset=ap_1d.offset,
                       ap=[[0, n]] + list(ap_1d.ap))

    with tile.TileContext(nc) as tc:
      with ExitStack() as stack:
        ps_pool = stack.enter_context(
            tc.tile_pool(name="ps", bufs=2, space="PSUM"))
        sm_pool = stack.enter_context(
            tc.tile_pool(name="sm", bufs=2, space="PSUM"))
        pj_pool = stack.enter_context(
            tc.tile_pool(name="pj", bufs=2, space="PSUM"))

        def psum(shape, dtype=F32):
            return ps_pool.tile(shape, dtype, tag="ps", name="pst")

        def psum_pj(shape, dtype=F32):
            return pj_pool.tile(shape, dtype, tag="pj", name="psj")

        def psum_sm(shape, dtype=F32):
            return sm_pool.tile(shape, dtype, tag="sm", name="pss")

        small = stack.enter_context(tc.tile_pool(name="small", bufs=1))
        ident = small.tile([P, P], BF16, name="ident")
        make_identity(nc, ident)
        eps_t = small.tile([P, 1], F32, name="eps_t")
        nc.vector.memset(eps_t, EPS)
        bq_sb = small.tile([P, DT], F32, name="bq_sb")
        nc.sync.dma_start(out=bq_sb, in_=bq_d.rearrange("(t p) -> p t", p=P))
        bk_sb = small.tile([P, DT], F32, name="bk_sb")
        nc.sync.dma_start(out=bk_sb, in_=bk_d.rearrange("(t p) -> p t", p=P))
        b1_sb = small.tile([P, FT], F32, name="b1_sb")
        nc.sync.dma_start(out=b1_sb, in_=b1_d.rearrange("(t p) -> p t", p=P))
        bv_bc = small.tile([P, D], BF16, name="bv_bc")
        nc.gpsimd.dma_start(out=bv_bc, in_=bcast(bv_d, P))
        bo_bc = small.tile([P, D], BF16, name="bo_bc")
        nc.gpsimd.dma_start(out=bo_bc, in_=bcast(bo_d, P))
        b2_bc = small.tile([P, D], BF16, name="b2_bc")
        nc.gpsimd.dma_start(out=b2_bc, in_=bcast(b2_d, P))

        # ---- right-side stack: LN1/QKV phase (released innermost-first) ----
        p_xnt = tc.alloc_tile_pool(name="p_xnt", bufs=1, side="right")
        xnt = p_xnt.tile([P, DT, TT_ALL, P], BF16, name="xnt")
        p_wv = tc.alloc_tile_pool(name="p_wv", bufs=1, side="right")
        wv_sb = p_wv.tile([P, DT, D], BF16, name="wv_sb")
        for dt in range(DT):
            nc.sync.dma_start(out=wv_sb[:, dt, :],
                              in_=wv_d[P * dt:P * (dt + 1), :])
        p_wk = tc.alloc_tile_pool(name="p_wk", bufs=1, side="right")
        wk_sb = p_wk.tile([P, DT, D], BF16, name="wk_sb")
        for dt in range(DT):
            nc.sync.dma_start(out=wk_sb[:, dt, :],
                              in_=wk_d[P * dt:P * (dt + 1), :])
        p_wq = tc.alloc_tile_pool(name="p_wq", bufs=1, side="right")
        wq_sb = p_wq.tile([P, DT, D], BF16, name="wq_sb")
        for dt in range(DT):
            nc.sync.dma_start(out=wq_sb[:, dt, :],
                              in_=wq_d[P * dt:P * (dt + 1), :])
        p_xall = tc.alloc_tile_pool(name="p_xall", bufs=1, side="right")
        x_all = p_xall.tile([P, TT_ALL, D], BF16, name="x_all")
        for tt in range(TT_ALL):
            nc.sync.dma_start(out=x_all[:, tt, :],
                              in_=xb_d[P * tt:P * (tt + 1), :])

        # ---------------- LN1 + transpose into xnt ----------------
        ln_pool = tc.alloc_tile_pool(name="ln_pool", bufs=3, side="right")
        for tt in range(TT_ALL):
            x_sl = x_all[:, tt, :]
            mean = ln_pool.tile([P, 1], F32, tag="mean", name="mean")
            var = ln_pool.tile([P, 1], F32, tag="var", name="var")
            if tt % 2 == 0:
                stats = ln_pool.tile([P, NG, 6], F32, tag="st", name="stats")
                for g in range(NG):
                    nc.vector.bn_stats(out=stats[:, g, :],
                                       in_=x_sl[:, GS * g:GS * (g + 1)])
                mv = ln_pool.tile([P, 2], F32, tag="mv", name="mv")
                nc.vector.bn_aggr(out=mv, in_=stats)
                nc.vector.tensor_copy(out=mean, in_=mv[:, 0:1])
                nc.vector.tensor_copy(out=var, in_=mv[:, 1:2])
            else:
                scr = ln_pool.tile([P, D], BF16, tag="scr", name="scr")
                s1 = ln_pool.tile([P, 1], F32, tag="s1", name="s1")
                ssq = ln_pool.tile([P, 1], F32, tag="ssq", name="ssq")
                nc.scalar.activation(out=scr, in_=x_sl, func=AF.Identity,
                                     accum_out=s1)
                nc.scalar.activation(out=scr, in_=x_sl, func=AF.Square,
                                     accum_out=ssq)
                nc.vector.tensor_scalar(out=mean, in0=s1, scalar1=1.0 / D,
                                        scalar2=None, op0=ALU.mult)
                m2 = ln_pool.tile([P, 1], F32, tag="m2", name="m2")
                nc.vector.tensor_tensor(out=m2, in0=mean, in1=mean,
                                        op=ALU.mult)
                nc.vector.tensor_scalar(out=var, in0=ssq, scalar1=1.0 / D,
                                        scalar2=None, op0=ALU.mult)
                nc.vector.tensor_tensor(out=var, in0=var, in1=m2,
                                        op=ALU.subtract)
            std = ln_pool.tile([P, 1], F32, tag="sd", name="std")
            nc.scalar.activation(out=std, in_=var, func=AF.Sqrt,
                                 bias=eps_t, scale=1.0)
            rstd = ln_pool.tile([P, 1], F32, tag="rs", name="rstd")
            nc.vector.reciprocal(out=rstd, in_=std)
            xn_t = ln_pool.tile([P, D], BF16, tag="xn", name="xn_t")
            if tt % 2 == 0:
                nc.vector.tensor_scalar(out=xn_t, in0=x_sl,
                                        scalar1=mean, scalar2=rstd,
                                        op0=ALU.subtract, op1=ALU.mult)
            else:
                nb = ln_pool.tile([P, 1], F32, tag="nb", name="nb")
                nc.vector.tensor_scalar(out=nb, in0=mean, scalar1=rstd,
                                        scalar2=-1.0, op0=ALU.mult,
                                        op1=ALU.mult)
                nc.scalar.activation(out=xn_t, in_=x_sl, func=AF.Identity,
                                     bias=nb, scale=rstd)
            for dt0 in range(0, DT, 4):
                g = min(4, DT - dt0)
                tp = psum_sm([P, g * P], BF16)
                for j in range(g):
                    nc.tensor.transpose(
                        tp[:, P * j:P * (j + 1)],
                        xn_t[:, P * (dt0 + j):P * (dt0 + j + 1)], ident)
                nc.vector.tensor_copy(out=xnt[:, dt0:dt0 + g, tt, :],
                                      in_=tp)
        ln_pool.release()
        p_xall.release()

        # ---- left-side stack: attention-lifetime tensors ----
        p_ctxt = tc.alloc_tile_pool(name="p_ctxt", bufs=1, side="left")
        ctxt = p_ctxt.tile([P, DT, T], BF16, name="ctxt")   # ctxT [d, tok]
        p_wo = tc.alloc_tile_pool(name="p_wo", bufs=1, side="left")
        wo_sb = p_wo.tile([P, DT, D], BF16, name="wo_sb")
        for dt in range(DT):
            nc.sync.dma_start(out=wo_sb[:, dt, :],
                              in_=wo_d[P * dt:P * (dt + 1), :])
        p_qt = tc.alloc_tile_pool(name="p_qt", bufs=1, side="left")
        qt = p_qt.tile([P, DT, T], BF16, name="qt")         # qT [dout, own tok]

        # ------------- Q projection (transposed output) -------------
        for dot in range(DT):
            for c in range(T // QC):
                ps = psum([P, QC])
                for dt in range(DT):
                    nc.tensor.matmul(
                        ps, wq_sb[:, dt, P * dot:P * (dot + 1)],
                        xnt[:, dt, (QC // P) * c:(QC // P) * (c + 1), :],
                        start=(dt == 0), stop=(dt == DT - 1))
                nc.vector.tensor_scalar(
                    out=qt[:, dot, QC * c:QC * (c + 1)], in0=ps,
                    scalar1=bq_sb[:, dot:dot + 1], scalar2=None,
                    op0=ALU.add)
        p_wq.release()

        p_kt = tc.alloc_tile_pool(name="p_kt", bufs=3, side="left")
        p_va = tc.alloc_tile_pool(name="p_va", bufs=1, side="left")
        v_aug = p_va.tile([P, TT_ALL, H, HD + 1], BF16, name="v_aug")
        nc.vector.memset(v_aug[:, :, :, HD:HD + 1], 1.0)

        def emit_kproj(dot):
            kt_t = p_kt.tile([P, S], BF16, tag="ktt", name="kt_t")
            tkc = min(512, S)
            tpc = tkc // P
            for c in range(S // tkc):
                ps = psum_pj([P, tkc])
                for dt in range(DT):
                    nc.tensor.matmul(
                        ps, wk_sb[:, dt, P * dot:P * (dot + 1)],
                        xnt[:, dt, tpc * c:tpc * (c + 1), :],
                        start=(dt == 0), stop=(dt == DT - 1))
                nc.vector.tensor_scalar(
                    out=kt_t[:, tkc * c:tkc * (c + 1)], in0=ps,
                    scalar1=bk_sb[:, dot:dot + 1], scalar2=None,
                    op0=ALU.add)
            return kt_t

        def emit_vproj(c, tts=None):
            hpc = NC_D // HD  # heads per chunk
            for tt in (range(TT_ALL) if tts is None else tts):
                ps = psum_pj([P, NC_D])
                for dt in range(DT):
                    nc.tensor.matmul(
                        ps, xnt[:, dt, tt, :],
                        wv_sb[:, dt, NC_D * c:NC_D * (c + 1)],
                        start=(dt == 0), stop=(dt == DT - 1))
                nc.vector.tensor_tensor(
                    out=v_aug[:, tt, hpc * c:hpc * (c + 1), 0:HD],
                    in0=ps, in1=bv_bc[:, NC_D * c:NC_D * (c + 1)], op=ALU.add)

        # ---------------- attention ----------------
        # Software-pipelined emission: scores+exp of chunk i are emitted
        # before the ctx block of chunk i-1, so ACT (the attention
        # bottleneck) always has the next chunk's exps ready to chew on
        # while PE runs the previous chunk's ctx matmuls.
        exp_pool = tc.alloc_tile_pool(name="exp_pool", bufs=4, side="left")
        ctx_pool = tc.alloc_tile_pool(name="ctx_pool", bufs=3, side="left")

        HT = max(TT_ALL // 2, 1)

        def emit_scores(h, qc, kt_t):
            po = HD * (h % HPD)
            dot = h // HPD
            halves = []
            for hf in range(TT_ALL // HT):
                expt = exp_pool.tile([P, HT, QC], BF16, tag="expt",
                                     name="expt")
                for j0 in range(0, HT, 2):
                    g = min(2, HT - j0)
                    ps = psum([P, 2 * QC])
                    for jj in range(g):
                        st = hf * HT + j0 + jj
                        nc.tensor.matmul(
                            ps[:, QC * jj:QC * (jj + 1)],
                            kt_t[po:po + HD, P * st:P * (st + 1)],
                            qt[po:po + HD, dot, QC * qc:QC * (qc + 1)],
                            start=True, stop=True)
                    nc.scalar.activation(
                        out=expt[:, j0:j0 + g, :], in_=ps[:, 0:g * QC],
                        func=AF.Exp, scale=float(HD) ** -0.5)
                halves.append(expt)
            return halves

        def emit_ctx(h, qc, halves):
            po = HD * (h % HPD)
            dot = h // HPD
            ctp = psum_sm([HD, QSUB * P], BF16)
            for k in range(QSUB):
                cps = psum_sm([P, HD + 1])
                for st in range(TT_ALL):
                    expt = halves[st // HT]
                    nc.tensor.matmul(
                        cps, expt[:, st % HT, P * k:P * (k + 1)],
                        v_aug[:, st, h, :],
                        start=(st == 0), stop=(st == TT_ALL - 1))
                rec = ctx_pool.tile([P, 1], F32, tag="rec", name="rec")
                nc.vector.reciprocal(out=rec, in_=cps[:, HD:HD + 1])
                csb = ctx_pool.tile([P, HD], BF16, tag="csb", name="csb")
                nc.vector.tensor_scalar(out=csb, in0=cps[:, 0:HD],
                                        scalar1=rec, scalar2=None,
                                        op0=ALU.mult)
                nc.tensor.transpose(ctp[:, P * k:P * (k + 1)], csb, ident)
            nc.vector.tensor_copy(
                out=ctxt[po:po + HD, dot, QC * qc:QC * (qc + 1)], in_=ctp)

        prev = None
        kt_t = None
        for h in range(H):
            if h % HPD == 0:
                kt_t = emit_kproj(h // HPD)  # kT d-tile for heads h, h+1
            if h == 0:
                emit_vproj(0)
            if DCH > 1 and h == H // 2 - 2:
                emit_vproj(1, range(0, TT_ALL // 2))
            if DCH > 1 and h == H // 2 - 1:
                emit_vproj(1, range(TT_ALL // 2, TT_ALL))
            for qc in range(NQC):
                halves = emit_scores(h, qc, kt_t)
                if prev is not None:
                    emit_ctx(*prev)
                prev = (h, qc, halves)
        emit_ctx(*prev)
        p_wk.release()
        p_wv.release()
        p_xnt.release()
        ctx_pool.release()
        exp_pool.release()
        p_va.release()
        p_kt.release()
        p_qt.release()

        # ---- right-side stack: FFN weights + x2 (+ x_own reload) ----
        p_x2 = tc.alloc_tile_pool(name="p_x2", bufs=1, side="right")
        x2 = p_x2.tile([P, TT_OWN, D], F32, name="x2")
        p_xor = tc.alloc_tile_pool(name="p_xor", bufs=1, side="right")
        x_own = p_xor.tile([P, TT_OWN, D], F32, name="x_own")
        for tt in range(TT_OWN):
            nc.sync.dma_start(out=x_own[:, tt, :],
                              in_=x_d[P * tt:P * (tt + 1), :])
        p_w2 = tc.alloc_tile_pool(name="p_w2", bufs=1, side="right")
        w2_sb = p_w2.tile([P, FT, D], BF16, name="w2_sb")
        for ft in range(FT):
            nc.sync.dma_start(out=w2_sb[:, ft, :],
                              in_=w2_d[P * ft:P * (ft + 1), :])
        out_pool = tc.alloc_tile_pool(name="out_pool", bufs=3, side="right")
        p_w1 = tc.alloc_tile_pool(name="p_w1", bufs=1, side="right")
        w1_sb = p_w1.tile([P, DT, FF], BF16, name="w1_sb")
        for dt in range(DT):
            nc.sync.dma_start(out=w1_sb[:, dt, :],
                              in_=w1_d[P * dt:P * (dt + 1), :])
        p_xn2t = tc.alloc_tile_pool(name="p_xn2t", bufs=1, side="right")
        xn2t = p_xn2t.tile([P, DT, TT_OWN, P], BF16, name="xn2t")
        ln2_pool = tc.alloc_tile_pool(name="ln2_pool", bufs=3, side="right")

        # ------- out-proj + residual, LN2 interleaved per token tile -------
        for tt in range(TT_OWN):
            for c in range(DCH):
                ps = psum([P, NC_D])
                for dt in range(DT):
                    nc.tensor.matmul(
                        ps, ctxt[:, dt, P * tt:P * (tt + 1)],
                        wo_sb[:, dt, NC_D * c:NC_D * (c + 1)],
                        start=(dt == 0), stop=(dt == DT - 1))
                sl = slice(NC_D * c, NC_D * (c + 1))
                nc.vector.tensor_tensor(out=x2[:, tt, sl], in0=ps,
                                        in1=x_own[:, tt, sl], op=ALU.add)
                nc.vector.tensor_tensor(out=x2[:, tt, sl], in0=x2[:, tt, sl],
                                        in1=bo_bc[:, sl], op=ALU.add)
            x_sl = x2[:, tt, :]
            mean = ln2_pool.tile([P, 1], F32, tag="mean", name="mean2")
            var = ln2_pool.tile([P, 1], F32, tag="var", name="var2")
            if tt % 2 == 0:
                stats = ln2_pool.tile([P, NG, 6], F32, tag="st", name="stats2")
                for g in range(NG):
                    nc.vector.bn_stats(out=stats[:, g, :],
                                       in_=x_sl[:, GS * g:GS * (g + 1)])
                mv = ln2_pool.tile([P, 2], F32, tag="mv", name="mv2")
                nc.vector.bn_aggr(out=mv, in_=stats)
                nc.vector.tensor_copy(out=mean, in_=mv[:, 0:1])
                nc.vector.tensor_copy(out=var, in_=mv[:, 1:2])
            else:
                scr = ln2_pool.tile([P, D], BF16, tag="scr", name="scr2")
                s1 = ln2_pool.tile([P, 1], F32, tag="s1", name="s12")
                ssq = ln2_pool.tile([P, 1], F32, tag="ssq", name="ssq2")
                nc.scalar.activation(out=scr, in_=x_sl, func=AF.Identity,
                                     accum_out=s1)
                nc.scalar.activation(out=scr, in_=x_sl, func=AF.Square,
                                     accum_out=ssq)
                nc.vector.tensor_scalar(out=mean, in0=s1, scalar1=1.0 / D,
                                        scalar2=None, op0=ALU.mult)
                m2 = ln2_pool.tile([P, 1], F32, tag="m2", name="m22")
                nc.vector.tensor_tensor(out=m2, in0=mean, in1=mean,
                                        op=ALU.mult)
                nc.vector.tensor_scalar(out=var, in0=ssq, scalar1=1.0 / D,
                                        scalar2=None, op0=ALU.mult)
                nc.vector.tensor_tensor(out=var, in0=var, in1=m2,
                                        op=ALU.subtract)
            std = ln2_pool.tile([P, 1], F32, tag="sd", name="std2")
            nc.scalar.activation(out=std, in_=var, func=AF.Sqrt,
                                 bias=eps_t, scale=1.0)
            rstd = ln2_pool.tile([P, 1], F32, tag="rs", name="rstd2")
            nc.vector.reciprocal(out=rstd, in_=std)
            xn_t = ln2_pool.tile([P, D], BF16, tag="xn", name="xn2_t")
            if tt % 2 == 0:
                nc.vector.tensor_scalar(out=xn_t, in0=x_sl,
                                        scalar1=mean, scalar2=rstd,
                                        op0=ALU.subtract, op1=ALU.mult)
            else:
                nb = ln2_pool.tile([P, 1], F32, tag="nb", name="nb2")
                nc.vector.tensor_scalar(out=nb, in0=mean, scalar1=rstd,
                                        scalar2=-1.0, op0=ALU.mult,
                                        op1=ALU.mult)
                nc.scalar.activation(out=xn_t, in_=x_sl, func=AF.Identity,
                                     bias=nb, scale=rstd)
            for dt0 in range(0, DT, 4):
                g = min(4, DT - dt0)
                tp = psum_sm([P, g * P], BF16)
                for j in range(g):
                    nc.tensor.transpose(
                        tp[:, P * j:P * (j + 1)],
                        xn_t[:, P * (dt0 + j):P * (dt0 + j + 1)], ident)
                nc.vector.tensor_copy(out=xn2t[:, dt0:dt0 + g, tt, :], in_=tp)
        ln2_pool.release()
        p_wo.release()
        p_ctxt.release()
        p_ht = tc.alloc_tile_pool(name="p_ht", bufs=1, side="left")
        ht = p_ht.tile([P, FT, T], BF16, name="ht")        # hT [ff, tok]

        # ---------------- FFN fc1 (transposed output) ----------------
        tkc = min(512, T)
        tpc = tkc // P

        def emit_fc2(tt, c):
            ps = psum_pj([P, NC_D])
            for ft in range(FT):
                nc.tensor.matmul(
                    ps, ht[:, ft, P * tt:P * (tt + 1)],
                    w2_sb[:, ft, NC_D * c:NC_D * (c + 1)],
                    start=(ft == 0), stop=(ft == FT - 1))
            o_sb = out_pool.tile([P, NC_D], F32, tag="osb", name="o_sb")
            sl = slice(NC_D * c, NC_D * (c + 1))
            nc.vector.tensor_tensor(out=o_sb, in0=ps,
                                    in1=x2[:, tt, sl], op=ALU.add)
            nc.vector.tensor_tensor(out=o_sb, in0=o_sb,
                                    in1=b2_bc[:, sl], op=ALU.add)
            nc.sync.dma_start(out=out_d[P * tt:P * (tt + 1), sl], in_=o_sb)

        for c in range(T // tkc):
            for ft in range(FT):
                ps = psum([P, tkc])
                for dt in range(DT):
                    nc.tensor.matmul(
                        ps, w1_sb[:, dt, P * ft:P * (ft + 1)],
                        xn2t[:, dt, tpc * c:tpc * (c + 1), :],
                        start=(dt == 0), stop=(dt == DT - 1))
                nc.scalar.activation(
                    out=ht[:, ft, tkc * c:tkc * (c + 1)], in_=ps,
                    func=gelu_af, bias=b1_sb[:, ft:ft + 1], scale=1.0)
            # fc2 for this token chunk's tiles (hT columns complete)
            for tt in range(c * tpc, (c + 1) * tpc):
                for co in range(DCH):
                    emit_fc2(tt, co)
        p_ht.release()
        p_xn2t.release()
        p_w1.release()
        out_pool.release()
        p_w2.release()
        p_xor.release()
        p_x2.release()
    nc.compile()
    return nc


def _fold_host(inputs):
    """Fold LN affine + biases into weights (fp32), cast weights to bf16."""
    f = {k: np.asarray(v, dtype=np.float32) for k, v in inputs.items()}
    g1, b1, g2, b2 = f["g1"], f["b1"], f["g2"], f["b2"]
    bf = lambda a: np.ascontiguousarray(a).astype(ml_dtypes.bfloat16)
    w = {
        "wq": bf(g1[:, None] * f["Wq"]),
        "wk": bf(g1[:, None] * f["Wk"]),
        "wv": bf(g1[:, None] * f["Wv"]),
        "wo": bf(f["Wo"]),
        "w1": bf(g2[:, None] * f["W1"]),
        "w2": bf(f["W2"]),
        "bq": np.ascontiguousarray(b1 @ f["Wq"] + f["bq"]),
        "bk": np.ascontiguousarray(b1 @ f["Wk"] + f["bk"]),
        "bv": np.ascontiguousarray(b1 @ f["Wv"] + f["bv"]),
        "bo": np.ascontiguousarray(f["bo"]),
        "b1": np.ascontiguousarray(b2 @ f["W1"] + f["bf1"]),
        "b2": np.ascontiguousarray(f["bf2"]),
    }
    return f, w


def kernel(**inputs):
    global LAST_EXEC_NS, LAST_RESULTS, LAST_NC
    import os

    from concourse.bass_utils import run_bass_kernel_spmd

    f, w = _fold_host(inputs)
    x = f["x"]
    B, S, D = x.shape
    T = S // 2
    nc = build_nc(S=S, T=T, D=D, H=H_FULL, FF=FF_FULL)
    LAST_NC = nc

    in_maps = []
    for c in range(N_CORES):
        b, half = c // 2, c % 2
        if half == 0:
            xb = x[b]
        else:
            xb = np.concatenate([x[b, T:], x[b, :T]], axis=0)
        m = {"x": np.ascontiguousarray(xb),
             "xb": np.ascontiguousarray(xb).astype(ml_dtypes.bfloat16)}
        m.update(w)
        in_maps.append(m)

    trace = bool(int(os.environ.get("KBENCH_TRACE", "0")))
    res = run_bass_kernel_spmd(nc, in_maps, list(range(N_CORES)), trace=trace)
    LAST_EXEC_NS = res.exec_time_ns
    LAST_RESULTS = res

    out = np.empty((B, S, D), dtype=np.float32)
    for c in range(N_CORES):
        b, half = c // 2, c % 2
        out[b, T * half:T * (half + 1)] = res.results[c]["out"]
    return out



# revision 41
# speedup vs baseline: 1.4161x; 1.4161x over previous
"""Fused transformer block (LN -> MHA -> LN -> FFN) on 8 TRN2 NeuronCores.

Sharding: core c handles batch (c // 2), token half (c % 2).  The host rolls
each batch's tokens so every core's "own" tokens are rows 0..T-1 of its x
input; K/V are computed for all S tokens locally (duplicated within the
pair), so the 8 cores are fully independent (no collectives).

Numerics: LayerNorm affine + all linear biases are folded into the weights
on the host (x's bias-added residual is precomputed host-side); matmuls run
in fp8e4 (e4m3) with fp32 PSUM accumulation using DoubleRow perf mode (two
k-tiles contracted per instruction).  Softmax skips max-subtraction
(|scores| <= ~4 for LN'd inputs) but applies a constant -1.5 shift
(softmax-invariant) so exp() stays below the fp8e4 inf threshold; the
denominator comes from a ones-column appended to V.

Scores trick: Wq/Wk output columns are permuted on the host so each head's
64 dims are split as (dims 0-31 -> partitions 32q..32q+31 of one 128-block,
dims 32-63 -> the matching partitions of the next 128-block).  Head-internal
permutation leaves q.k unchanged, and the two half-blocks land in free-dim
position 1 of the qt/kt tiles -- exactly the [32, 2, N] operand layout
DoubleRow needs, so even the 64-deep scores contraction runs at 0.5
cycles/row.

exp() is elementwise-affine in fp8-byte space (exp(x) bitcast trick), so it
is spread across ACT (exact exp + convert), DVE and Pool (tensor_scalar
affine -> uint8 -> bitcast fp8) to keep all three engines under the PE
roofline.

Schedule: query-chunk-outer / head-inner attention; out-proj + LN2 + the
xn2t transposes for the first token half are emitted inside the second
query-chunk's head loop so they fill engine gaps instead of forming a
serial phase.  All 512-wide fp32 PSUMs share one 6-deep rotation; the ctx
accumulators + ctx transposes share a 2-deep rotation (8 banks total).
"""

from contextlib import ExitStack

import ml_dtypes
import numpy as np

import concourse.bass as bass
import concourse.mybir as mybir
import concourse.tile as tile
from concourse import bacc
from concourse.masks import make_identity

F32 = mybir.dt.float32
BF16 = mybir.dt.bfloat16
FP8 = mybir.dt.float8e4
U8 = mybir.dt.uint8
AF = mybir.ActivationFunctionType
ALU = mybir.AluOpType
DR = mybir.MatmulPerfMode.DoubleRow

B_FULL = 4
S_FULL = 2048
D_FULL = 1024
H_FULL = 16
FF_FULL = 2048
HD = 64
EPS = 1e-5
N_CORES = 8

# softmax constants (scores scale 1/8, constant shift -1.5)
SM_SCALE = float(HD) ** -0.5
SM_SHIFT = -1.5
# fast-exp affine in e4m3 byte space: byte = s*K8 + B8
K8 = SM_SCALE * 8.0 * np.log2(np.e)
B8 = 7 * 8 + SM_SHIFT * 8.0 * np.log2(np.e)

# exp engine schedule, cycled per exp-instruction: A=ACT exact, D=DVE fast,
# P=Pool fast
EXP_PAT = "ADADADAD"

LAST_EXEC_NS = None
LAST_RESULTS = None
LAST_NC = None


def build_nc(S=S_FULL, T=S_FULL // 2, D=D_FULL, H=H_FULL, FF=FF_FULL,
             gelu_af=AF.Gelu, zero_bv=False, zero_b2=False):
    """Build the single-core (SPMD) Bass program.

    S: total tokens per batch (K/V length), T: own tokens (Q length),
    D: model dim, H: heads (H*64 == D), FF: hidden dim.
    """
    assert H * HD == D
    P = 128
    DT = D // P           # d-tiles (contraction tiles over D)
    TT_ALL = S // P       # token tiles over full sequence
    TT_OWN = T // P       # token tiles over own tokens
    FT = FF // P          # ff tiles
    QC = min(512, T)      # q chunk (columns per scores matmul)
    NQC = T // QC
    QSUB = QC // P
    NC_D = min(512, D)    # matmul N chunk over D
    DCH = D // NC_D
    NG = 2                # bn_stats groups
    GS = D // NG
    NHG = H // 4          # head groups of 4 (one [128,2,S] kt tile each)

    nc = bacc.Bacc("TRN2", target_bir_lowering=False, debug=False,
                   enable_asserts=False, num_devices=N_CORES)

    xpb_d = nc.dram_tensor("xpb", [T, D], F32, kind="ExternalInput").ap()
    xb_d = nc.dram_tensor("xb", [S, D], FP8, kind="ExternalInput").ap()
    wq_d = nc.dram_tensor("wq", [D, D], FP8, kind="ExternalInput").ap()
    wk_d = nc.dram_tensor("wk", [D, D], FP8, kind="ExternalInput").ap()
    wv_d = nc.dram_tensor("wv", [D, D], FP8, kind="ExternalInput").ap()
    wo_d = nc.dram_tensor("wo", [D, D], FP8, kind="ExternalInput").ap()
    w1_d = nc.dram_tensor("w1", [D, FF], FP8, kind="ExternalInput").ap()
    w2_d = nc.dram_tensor("w2", [FF, D], FP8, kind="ExternalInput").ap()
    bq_d = nc.dram_tensor("bq", [D], F32, kind="ExternalInput").ap()
    bk_d = nc.dram_tensor("bk", [D], F32, kind="ExternalInput").ap()
    bv_d = nc.dram_tensor("bv", [D], F32, kind="ExternalInput").ap()
    b1_d = nc.dram_tensor("b1", [FF], F32, kind="ExternalInput").ap()
    b2_d = nc.dram_tensor("b2", [D], F32, kind="ExternalInput").ap()
    out_d = nc.dram_tensor("out", [T, D], F32, kind="ExternalOutput").ap()

    def bcast(ap_1d, n):
        return bass.AP(tensor=ap_1d.tensor, offset=ap_1d.offset,
                       ap=[[0, n]] + list(ap_1d.ap))

    exp_idx = [0]
    cp_idx = [0]
    expt_bufs = 4 if (zero_bv and zero_b2) else 3

    with tile.TileContext(nc) as tc:
      with ExitStack() as stack:
        ps_pool = stack.enter_context(
            tc.tile_pool(name="ps", bufs=1, space="PSUM"))

        def psum(shape, dtype=F32):
            return ps_pool.tile(shape, dtype, tag="sc", name="pst", bufs=3)

        def psum_ctx(shape, dtype=F32):
            return ps_pool.tile(shape, dtype, tag="ps4", name="ps4", bufs=2)

        small = stack.enter_context(tc.tile_pool(name="small", bufs=1))
        ident = small.tile([P, P], BF16, name="ident")
        make_identity(nc, ident)
        eps_t = small.tile([P, 1], F32, name="eps_t")
        nc.vector.memset(eps_t, EPS)
        shift_t = small.tile([P, 1], F32, name="shift_t")
        nc.vector.memset(shift_t, SM_SHIFT)
        bq_sb = small.tile([P, DT], F32, name="bq_sb")
        nc.sync.dma_start(out=bq_sb, in_=bq_d.rearrange("(t p) -> p t", p=P))
        bk_sb = small.tile([P, DT], F32, name="bk_sb")
        nc.sync.dma_start(out=bk_sb, in_=bk_d.rearrange("(t p) -> p t", p=P))
        b1_sb = small.tile([P, FT], F32, name="b1_sb")
        nc.sync.dma_start(out=b1_sb, in_=b1_d.rearrange("(t p) -> p t", p=P))
        if not zero_bv:
            bv_bc = small.tile([P, D], F32, name="bv_bc")
            nc.gpsimd.dma_start(out=bv_bc, in_=bcast(bv_d, P))
        if not zero_b2:
            b2_bc = small.tile([P, D], F32, name="b2_bc")
            nc.gpsimd.dma_start(out=b2_bc, in_=bcast(b2_d, P))

        # ---- right-side stack bottom: tensors that survive into the FFN ----
        p_w1 = tc.alloc_tile_pool(name="p_w1", bufs=1, side="right")
        w1_sb = p_w1.tile([P, DT, FF], FP8, name="w1_sb")
        p_w2 = tc.alloc_tile_pool(name="p_w2", bufs=1, side="right")
        w2_sb = p_w2.tile([P, FT, D], FP8, name="w2_sb")
        p_ht = tc.alloc_tile_pool(name="p_ht", bufs=1, side="right")
        ht = p_ht.tile([P, FT, T], FP8, name="ht")        # hT [ff, tok]
        p_x2 = tc.alloc_tile_pool(name="p_x2", bufs=1, side="right")
        x2 = p_x2.tile([P, TT_OWN, D], F32, name="x2")
        p_xn2t = tc.alloc_tile_pool(name="p_xn2t", bufs=1, side="right")
        xn2t = p_xn2t.tile([P, DT, TT_OWN, P], FP8, name="xn2t")

        # ---- right-side stack: LN1/QKV phase (released innermost-first) ----
        p_xnt = tc.alloc_tile_pool(name="p_xnt", bufs=1, side="right")
        xnt = p_xnt.tile([P, DT, TT_ALL, P], FP8, name="xnt")
        p_wv = tc.alloc_tile_pool(name="p_wv", bufs=1, side="right")
        wv_sb = p_wv.tile([P, DT, D], FP8, name="wv_sb")
        p_wk = tc.alloc_tile_pool(name="p_wk", bufs=1, side="right")
        wk_sb = p_wk.tile([P, DT, D], FP8, name="wk_sb")
        p_wq = tc.alloc_tile_pool(name="p_wq", bufs=1, side="right")
        wq_sb = p_wq.tile([P, DT, D], FP8, name="wq_sb")
        p_xall = tc.alloc_tile_pool(name="p_xall", bufs=1, side="right")
        x_all = p_xall.tile([P, TT_ALL, D], FP8, name="x_all")
        # x_all first (LN1 is the first consumer), then Q/K/V weights
        for tt in range(TT_ALL):
            nc.sync.dma_start(out=x_all[:, tt, :],
                              in_=xb_d[P * tt:P * (tt + 1), :])
        for dt in range(DT):
            nc.sync.dma_start(out=wq_sb[:, dt, :],
                              in_=wq_d[P * dt:P * (dt + 1), :])
        for dt in range(DT):
            nc.sync.dma_start(out=wk_sb[:, dt, :],
                              in_=wk_d[P * dt:P * (dt + 1), :])
        for dt in range(DT):
            nc.sync.dma_start(out=wv_sb[:, dt, :],
                              in_=wv_d[P * dt:P * (dt + 1), :])
        for dt in range(DT):
            nc.sync.dma_start(out=w1_sb[:, dt, :],
                              in_=w1_d[P * dt:P * (dt + 1), :])
        for ft in range(FT):
            nc.sync.dma_start(out=w2_sb[:, ft, :],
                              in_=w2_d[P * ft:P * (ft + 1), :])

        def emit_ln(pool, x_sl, tt, xn_t):
            """LayerNorm stats + normalized write into bf16 xn_t.
            Alternates DVE (bn_stats) and ACT (accum) paths per token tile."""
            act_path = tt % 2 == 1
            mean = pool.tile([P, 1], F32, tag="mean", name="mean")
            var = pool.tile([P, 1], F32, tag="var", name="var")
            if not act_path:
                stats = pool.tile([P, NG, 6], F32, tag="st", name="stats")
                for g in range(NG):
                    nc.vector.bn_stats(out=stats[:, g, :],
                                       in_=x_sl[:, GS * g:GS * (g + 1)])
                mv = pool.tile([P, 2], F32, tag="mv", name="mv")
                nc.vector.bn_aggr(out=mv, in_=stats)
                nc.vector.tensor_copy(out=mean, in_=mv[:, 0:1])
                nc.vector.tensor_copy(out=var, in_=mv[:, 1:2])
            else:
                scr = pool.tile([P, D], BF16, tag="scr", name="scr")
                s1 = pool.tile([P, 1], F32, tag="s1", name="s1")
                ssq = pool.tile([P, 1], F32, tag="ssq", name="ssq")
                nc.scalar.activation(out=scr, in_=x_sl, func=AF.Identity,
                                     accum_out=s1)
                nc.scalar.activation(out=scr, in_=x_sl, func=AF.Square,
                                     accum_out=ssq)
                nc.vector.tensor_scalar(out=mean, in0=s1, scalar1=1.0 / D,
                                        scalar2=None, op0=ALU.mult)
                m2 = pool.tile([P, 1], F32, tag="m2", name="m2")
                nc.vector.tensor_tensor(out=m2, in0=mean, in1=mean,
                                        op=ALU.mult)
                nc.vector.tensor_scalar(out=var, in0=ssq, scalar1=1.0 / D,
                                        scalar2=None, op0=ALU.mult)
                nc.vector.tensor_tensor(out=var, in0=var, in1=m2,
                                        op=ALU.subtract)
            std = pool.tile([P, 1], F32, tag="sd", name="std")
            nc.scalar.activation(out=std, in_=var, func=AF.Sqrt,
                                 bias=eps_t, scale=1.0)
            rstd = pool.tile([P, 1], F32, tag="rs", name="rstd")
            nc.vector.reciprocal(out=rstd, in_=std)
            if tt % 2 == 0:
                nc.gpsimd.tensor_scalar(out=xn_t, in0=x_sl,
                                        scalar1=mean, scalar2=rstd,
                                        op0=ALU.subtract, op1=ALU.mult)
            else:
                nb = pool.tile([P, 1], F32, tag="nb", name="nb")
                nc.vector.tensor_scalar(out=nb, in0=mean, scalar1=rstd,
                                        scalar2=-1.0, op0=ALU.mult,
                                        op1=ALU.mult)
                nc.scalar.activation(out=xn_t, in_=x_sl, func=AF.Identity,
                                     bias=nb, scale=rstd)

        def emit_transposes(xn_t, dst, tt):
            """PE-transpose bf16 xn_t into fp8 dst[:, :, tt, :]; one batched
            psum -> one SBUF copy, engine round-robined."""
            tp = psum([P, DT * P], BF16)
            for j in range(DT):
                nc.tensor.transpose(
                    tp[:, P * j:P * (j + 1)],
                    xn_t[:, P * j:P * (j + 1)], ident)
            which = cp_idx[0] % 2
            cp_idx[0] += 1
            if which == 0:
                nc.vector.tensor_copy(out=dst[:, :, tt, :], in_=tp)
            else:
                nc.scalar.activation(out=dst[:, :, tt, :], in_=tp,
                                     func=AF.Identity)

        # ---------------- LN1 + transpose into xnt ----------------
        ln_pool = tc.alloc_tile_pool(name="ln_pool", bufs=5, side="right")
        for tt in range(TT_ALL):
            xn_t = ln_pool.tile([P, D], BF16, tag="xn", name="xn_t")
            emit_ln(ln_pool, x_all[:, tt, :], tt, xn_t)
            emit_transposes(xn_t, xnt, tt)
        ln_pool.release()
        p_xall.release()

        # ---- left-side stack: attention-lifetime tensors ----
        p_ctxt = tc.alloc_tile_pool(name="p_ctxt", bufs=1, side="left")
        ctxt = p_ctxt.tile([P, DT, T], FP8, name="ctxt")   # ctxT [d, tok]
        p_wo = tc.alloc_tile_pool(name="p_wo", bufs=1, side="left")
        wo_sb = p_wo.tile([P, DT, D], FP8, name="wo_sb")
        for dt in range(DT):
            nc.sync.dma_start(out=wo_sb[:, dt, :],
                              in_=wo_d[P * dt:P * (dt + 1), :])
        p_qt = tc.alloc_tile_pool(name="p_qt", bufs=1, side="left")
        # qT in scores layout: [32q.., g, half, tok]
        qt = p_qt.tile([P, NHG, 2, T], FP8, name="qt")

        # ------------- Q projection (transposed output) -------------
        # permuted block b holds (head-group b//2, dim-half b%2)
        QPC = min(1024, T)
        for b in range(DT):
            for c in range(T // QPC):
                ps = psum([P, QPC])
                for j in range(QPC // 512):
                    t0 = (QPC * c + 512 * j) // P
                    for dt in range(0, DT, 2):
                        nc.tensor.matmul(
                            ps[:, 512 * j:512 * (j + 1)],
                            wq_sb[:, dt:dt + 2, P * b:P * (b + 1)],
                            xnt[:, dt:dt + 2, t0:t0 + 4, :],
                            start=(dt == 0), stop=(dt == DT - 2),
                            perf_mode=DR)
                qdst = qt[:, b // 2, b % 2, QPC * c:QPC * (c + 1)]
                if b % 2 == 0:
                    nc.scalar.activation(out=qdst, in_=ps, func=AF.Identity,
                                         bias=bq_sb[:, b:b + 1])
                else:
                    nc.vector.tensor_scalar(out=qdst, in0=ps,
                                            scalar1=bq_sb[:, b:b + 1],
                                            scalar2=None, op0=ALU.add)
        p_wq.release()

        ln2_pool = tc.alloc_tile_pool(name="ln2_pool", bufs=3, side="right")
        p_kt = tc.alloc_tile_pool(name="p_kt", bufs=4, side="left")
        p_va = tc.alloc_tile_pool(name="p_va", bufs=1, side="left")
        v_aug = p_va.tile([P, TT_ALL, H, HD + 1], FP8, name="v_aug")
        nc.vector.memset(v_aug[:, :, :, HD:HD + 1], 1.0)

        def emit_kproj(g, kt_t=None, parts=None):
            """kT for head group g: [128, 2, S] (partitions 32q hold head
            4g+q; free dim 1 holds the two 32-dim halves).  `parts` selects a
            subset of (half, chunk) pieces so emission can be spread."""
            if kt_t is None:
                kt_t = p_kt.tile([P, 2, S], FP8, tag="ktt", name="kt_t")
            tkc = min(1024, S)
            tpc = tkc // P
            for half in range(2):
                b = 2 * g + half
                for c in range(S // tkc):
                    if parts is not None and (half, c) not in parts:
                        continue
                    ps = psum([P, tkc])
                    for j in range(tkc // 512):
                        t0 = tpc * c + 4 * j
                        for dt in range(0, DT, 2):
                            nc.tensor.matmul(
                                ps[:, 512 * j:512 * (j + 1)],
                                wk_sb[:, dt:dt + 2, P * b:P * (b + 1)],
                                xnt[:, dt:dt + 2, t0:t0 + 4, :],
                                start=(dt == 0), stop=(dt == DT - 2),
                                perf_mode=DR)
                    kdst = kt_t[:, half, tkc * c:tkc * (c + 1)]
                    if (half + c) % 2 == 0:
                        nc.vector.tensor_scalar(out=kdst, in0=ps,
                                                scalar1=bk_sb[:, b:b + 1],
                                                scalar2=None, op0=ALU.add)
                    else:
                        nc.scalar.activation(out=kdst, in_=ps,
                                             func=AF.Identity,
                                             bias=bk_sb[:, b:b + 1])
            return kt_t

        def emit_vproj(tts):
            for tt in tts:
                ps = psum([P, D])
                for j in range(D // 512):
                    for dt in range(0, DT, 2):
                        nc.tensor.matmul(
                            ps[:, 512 * j:512 * (j + 1)],
                            xnt[:, dt:dt + 2, tt, :],
                            wv_sb[:, dt:dt + 2, 512 * j:512 * (j + 1)],
                            start=(dt == 0), stop=(dt == DT - 2),
                            perf_mode=DR)
                dst = v_aug[:, tt, :, 0:HD]
                if not zero_bv:
                    nc.vector.tensor_tensor(out=dst, in0=ps, in1=bv_bc,
                                            op=ALU.add)
                elif tt % 2 == 0:
                    nc.scalar.activation(out=dst, in_=ps, func=AF.Identity)
                else:
                    nc.vector.tensor_copy(out=dst, in_=ps)


        def fetch_xpb(tt):
            # residual lands directly in x2; out-proj accumulates in place
            nc.sync.dma_start(out=x2[:, tt, :],
                              in_=xpb_d[P * tt:P * (tt + 1), :])

        def emit_outproj(tt):
            """out-proj + residual for token tile tt."""
            ps = psum([P, D])
            for j in range(D // 512):
                for dt in range(0, DT, 2):
                    nc.tensor.matmul(
                        ps[:, 512 * j:512 * (j + 1)],
                        ctxt[:, dt:dt + 2, P * tt:P * (tt + 1)],
                        wo_sb[:, dt:dt + 2, 512 * j:512 * (j + 1)],
                        start=(dt == 0), stop=(dt == DT - 2), perf_mode=DR)
            nc.vector.tensor_tensor(out=x2[:, tt, :], in0=ps,
                                    in1=x2[:, tt, :], op=ALU.add)

        def emit_ln2(tt):
            """LN2 + xn2t transposes for token tile tt."""
            xn_t = ln2_pool.tile([P, D], BF16, tag="xn", name="xn2_t")
            emit_ln(ln2_pool, x2[:, tt, :], tt, xn_t)
            emit_transposes(xn_t, xn2t, tt)

        def emit_outproj_ln2(tt):
            emit_outproj(tt)
            emit_ln2(tt)

        tkc = min(256, T)
        tpc = tkc // P

        def emit_fc1(c, ft0):
            if True:
                ps = psum([P, 2 * tkc])
                for j in range(2):
                    ft = ft0 + j
                    for dt in range(0, DT, 2):
                        nc.tensor.matmul(
                            ps[:, tkc * j:tkc * (j + 1)],
                            w1_sb[:, dt:dt + 2, P * ft:P * (ft + 1)],
                            xn2t[:, dt:dt + 2, tpc * c:tpc * (c + 1), :],
                            start=(dt == 0), stop=(dt == DT - 2),
                            perf_mode=DR)
                # per-partition bias differs between the two ft blocks only
                # via b1_sb columns; gelu is emitted per block to keep the
                # bias correct but reads the shared psum
                for j in range(2):
                    ft = ft0 + j
                    nc.scalar.activation(
                        out=ht[:, ft, tkc * c:tkc * (c + 1)],
                        in_=ps[:, tkc * j:tkc * (j + 1)],
                        func=gelu_af, bias=b1_sb[:, ft:ft + 1], scale=1.0)

        def emit_fc2(tt):
            ps = psum([P, D])
            for j in range(D // 512):
                for ft in range(0, FT, 2):
                    nc.tensor.matmul(
                        ps[:, 512 * j:512 * (j + 1)],
                        ht[:, ft:ft + 2, P * tt:P * (tt + 1)],
                        w2_sb[:, ft:ft + 2, 512 * j:512 * (j + 1)],
                        start=(ft == 0), stop=(ft == FT - 2), perf_mode=DR)
            # x2[:, tt, :] is dead after this add: accumulate the final
            # output in place and DMA straight from it
            nc.vector.tensor_tensor(out=x2[:, tt, :], in0=ps,
                                    in1=x2[:, tt, :], op=ALU.add)
            if not zero_b2:
                nc.vector.tensor_tensor(out=x2[:, tt, :], in0=x2[:, tt, :],
                                        in1=b2_bc, op=ALU.add)
            nc.sync.dma_start(out=out_d[P * tt:P * (tt + 1), :],
                              in_=x2[:, tt, :])

        # ---------------- attention ----------------
        # Query-chunk-outer / head-inner; software-pipelined so scores+exp of
        # chunk i are emitted before the ctx block of chunk i-1.  During the
        # second query chunk, out-proj/LN2 work for the first chunk's tokens
        # is woven between head iterations.
        exp_pool = tc.alloc_tile_pool(name="exp_pool", bufs=1, side="left")
        ctx_pool = tc.alloc_tile_pool(name="ctx_pool", bufs=3, side="left")

        HT = TT_ALL // 2

        def emit_exp(ps, dst):
            eng = EXP_PAT[exp_idx[0] % len(EXP_PAT)]
            exp_idx[0] += 1
            if eng == "A":
                nc.scalar.activation(out=dst, in_=ps, func=AF.Exp,
                                     scale=SM_SCALE, bias=shift_t)
            elif eng == "D":
                nc.vector.tensor_scalar(out=dst.bitcast(U8), in0=ps,
                                        scalar1=float(K8), scalar2=float(B8),
                                        op0=ALU.mult, op1=ALU.add)
            else:
                nc.gpsimd.tensor_scalar(out=dst.bitcast(U8), in0=ps,
                                        scalar1=float(K8), scalar2=float(B8),
                                        op0=ALU.mult, op1=ALU.add)

        def emit_scores(h, qc, kt_t):
            g, q = h // 4, h % 4
            po = 32 * q
            halves = []
            for hf in range(2):
                expt = exp_pool.tile([P, HT, QC], FP8, tag="expt",
                                     name="expt", bufs=expt_bufs)
                for j0 in range(0, HT, 2):
                    ps = psum([P, 2 * QC])
                    for jj in range(2):
                        st = hf * HT + j0 + jj
                        nc.tensor.matmul(
                            ps[:, QC * jj:QC * (jj + 1)],
                            kt_t[po:po + 32, :, P * st:P * (st + 1)],
                            qt[po:po + 32, g, :, QC * qc:QC * (qc + 1)],
                            start=True, stop=True, perf_mode=DR,
                            tile_position=(po, 0))
                    emit_exp(ps, expt[:, j0:j0 + 2, :])
                halves.append(expt)
            return halves

        def emit_ctx(h, qc, halves):
            po = HD * (h % 2)
            dot = h // 2
            ctp = ps_pool.tile([HD, QSUB * P], BF16, tag="ps4", name="ctp",
                               bufs=2)
            ps4 = psum_ctx([P, QSUB, HD + 1])
            for k in range(QSUB):
                for st0 in range(0, TT_ALL, 2):
                    expt = halves[st0 // HT]
                    nc.tensor.matmul(
                        ps4[:, k, :],
                        expt[:, st0 % HT:st0 % HT + 2, P * k:P * (k + 1)],
                        v_aug[:, st0:st0 + 2, h, :],
                        start=(st0 == 0), stop=(st0 == TT_ALL - 2),
                        perf_mode=DR)
            rec = ctx_pool.tile([P, QSUB], F32, tag="rec", name="rec")
            nc.vector.reciprocal(out=rec, in_=ps4[:, :, HD])
            for k in range(QSUB):
                csb = ctx_pool.tile([P, HD], BF16, tag="csb", name="csb",
                                    bufs=6)
                nc.scalar.activation(out=csb, in_=ps4[:, k, 0:HD],
                                     func=AF.Identity,
                                     scale=rec[:, k:k + 1])
                nc.tensor.transpose(ctp[:, P * k:P * (k + 1)], csb, ident)
            nc.vector.tensor_copy(
                out=ctxt[po:po + HD, dot, QC * qc:QC * (qc + 1)], in_=ctp)

        for tt in range(min(2, TT_OWN)):
            fetch_xpb(tt)

        kt_ts = [emit_kproj(0)]
        prev = None
        for qc in range(NQC):
            for h in range(H):
                if qc == 0:
                    g_next, piece = h // 4 + 1, h % 4
                    if g_next < NHG:
                        if piece == 0:
                            kt_ts.append(emit_kproj(
                                g_next, parts=[(0, 0)]))
                        else:
                            emit_kproj(g_next, kt_t=kt_ts[g_next],
                                       parts=[(piece // 2, piece % 2)])
                    if h == 0:
                        emit_vproj(range(0, TT_ALL // 2))
                    if h == 1:
                        emit_vproj(range(TT_ALL // 2, TT_ALL))
                elif h >= 2 and (h - 2) // 4 < QSUB:
                    # weave first-half out-proj/LN2 + the FFN pipeline for
                    # already-finished token pairs between head iterations,
                    # one small piece per head iteration
                    tt, piece = (h - 2) // 4, (h - 2) % 4
                    if piece == 0:
                        if tt + 2 < TT_OWN:
                            fetch_xpb(tt + 2)
                        emit_outproj(tt)
                    elif piece == 1:
                        emit_ln2(tt)
                    elif tt % 2 == 1:
                        ch = tt // 2      # token pair (2ch, 2ch+1) done
                        fh = range(0, FT // 2, 2) if piece == 2 else \
                            range(FT // 2, FT, 2)
                        for ft0 in fh:
                            emit_fc1(ch, ft0)
                        if ch == 1 and piece == 3:
                            emit_fc2(0)
                            emit_fc2(1)
                if prev is not None:
                    emit_ctx(*prev)
                prev = (h, qc, emit_scores(h, qc, kt_ts[h // 4]))
        emit_ctx(*prev)
        for tt in range(QSUB, TT_OWN):
            if tt + 2 < TT_OWN:
                fetch_xpb(tt + 2)
            emit_outproj_ln2(tt)
            if tt == QSUB:
                # token pair (2,3)'s fc1 didn't fit in the weave window
                for ft0 in range(0, FT, 2):
                    emit_fc1(1, ft0)
                emit_fc2(0)
                emit_fc2(1)
            if tt % 2 == 1:
                ch = tt // 2
                for ft0 in range(0, FT, 2):
                    emit_fc1(ch, ft0)
                emit_fc2(2 * ch - 2)
                emit_fc2(2 * ch - 1)
        emit_fc2(TT_OWN - 2)
        emit_fc2(TT_OWN - 1)
        ln2_pool.release()
        ctx_pool.release()
        exp_pool.release()
        p_va.release()
        p_kt.release()
        p_qt.release()
        p_wo.release()
        p_ctxt.release()
        p_wk.release()
        p_wv.release()
        p_xnt.release()

        # ---------------- FFN ----------------



        p_xn2t.release()
        p_x2.release()
        p_ht.release()
        p_w2.release()
        p_w1.release()
    nc.compile()
    return nc


def _qk_perm(D=D_FULL):
    """Column permutation for Wq/Wk: block b holds (head-group b//2,
    dim-half b%2); partitions 32q..32q+31 of a block hold head 4*(b//2)+q."""
    perm = np.empty(D, dtype=np.int64)
    for p_col in range(D):
        b, p = divmod(p_col, 128)
        g, half = divmod(b, 2)
        head = 4 * g + p // 32
        dim = 32 * half + p % 32
        perm[p_col] = 64 * head + dim
    return perm


def _fold_host(inputs):
    """Fold LN affine + biases into weights (fp32), permute Q/K columns for
    the DoubleRow scores layout, cast weights to fp8e4 (e4m3)."""
    f = {k: np.asarray(v, dtype=np.float32) for k, v in inputs.items()}
    g1, b1, g2, b2 = f["g1"], f["b1"], f["g2"], f["b2"]
    perm = _qk_perm(f["Wq"].shape[0])
    f8 = lambda a: np.ascontiguousarray(a).astype(ml_dtypes.float8_e4m3)
    w = {
        "wq": f8((g1[:, None] * f["Wq"])[:, perm]),
        "wk": f8((g1[:, None] * f["Wk"])[:, perm]),
        "wv": f8(g1[:, None] * f["Wv"]),
        "wo": f8(f["Wo"]),
        "w1": f8(g2[:, None] * f["W1"]),
        "w2": f8(f["W2"]),
        "bq": np.ascontiguousarray((b1 @ f["Wq"] + f["bq"])[perm]),
        "bk": np.ascontiguousarray((b1 @ f["Wk"] + f["bk"])[perm]),
        "bv": np.ascontiguousarray(f["bv"]),
        "b1": np.ascontiguousarray(b2 @ f["W1"] + f["bf1"]),
        "b2": np.ascontiguousarray(f["bf2"]),
    }
    return f, w


def kernel(**inputs):
    global LAST_EXEC_NS, LAST_RESULTS, LAST_NC
    import os

    from concourse.bass_utils import run_bass_kernel_spmd

    f, w = _fold_host(inputs)
    x = f["x"]
    B, S, D = x.shape
    T = S // 2
    zero_bv = not np.any(w["bv"])
    zero_b2 = not np.any(w["b2"])
    nc = build_nc(S=S, T=T, D=D, H=H_FULL, FF=FF_FULL,
                  zero_bv=zero_bv, zero_b2=zero_b2)
    LAST_NC = nc

    in_maps = []
    for c in range(N_CORES):
        b, half = c // 2, c % 2
        if half == 0:
            xb = x[b]
        else:
            xb = np.concatenate([x[b, T:], x[b, :T]], axis=0)
        m = {"xpb": np.ascontiguousarray(xb[:T] + f["bo"][None, :]),
             "xb": np.ascontiguousarray(xb).astype(ml_dtypes.float8_e4m3)}
        m.update(w)
        in_maps.append(m)

    trace = bool(int(os.environ.get("KBENCH_TRACE", "0")))
    res = run_bass_kernel_spmd(nc, in_maps, list(range(N_CORES)), trace=trace)
    LAST_EXEC_NS = res.exec_time_ns
    LAST_RESULTS = res

    out = np.empty((B, S, D), dtype=np.float32)
    for c in range(N_CORES):
        b, half = c // 2, c % 2
        out[b, T * half:T * (half + 1)] = res.results[c]["out"]
    return out


# revision 46
# speedup vs baseline: 1.4169x; 1.0005x over previous
"""Fused transformer block (LN -> MHA -> LN -> FFN) on 8 TRN2 NeuronCores.

Sharding: core c handles batch (c // 2), token half (c % 2).  The host rolls
each batch's tokens so every core's "own" tokens are rows 0..T-1 of its x
input; K/V are computed for all S tokens locally (duplicated within the
pair), so the 8 cores are fully independent (no collectives).

Numerics: LayerNorm affine + all linear biases are folded into the weights
on the host (x's bias-added residual is precomputed host-side); matmuls run
in fp8e4 (e4m3) with fp32 PSUM accumulation using DoubleRow perf mode (two
k-tiles contracted per instruction).  Softmax skips max-subtraction
(|scores| <= ~4 for LN'd inputs) but applies a constant -1.5 shift
(softmax-invariant) so exp() stays below the fp8e4 inf threshold; the
denominator comes from a ones-column appended to V.

Scores trick: Wq/Wk output columns are permuted on the host so each head's
64 dims are split as (dims 0-31 -> partitions 32q..32q+31 of one 128-block,
dims 32-63 -> the matching partitions of the next 128-block).  Head-internal
permutation leaves q.k unchanged, and the two half-blocks land in free-dim
position 1 of the qt/kt tiles -- exactly the [32, 2, N] operand layout
DoubleRow needs, so even the 64-deep scores contraction runs at 0.5
cycles/row.

exp() is elementwise-affine in fp8-byte space (exp(x) bitcast trick), so it
is spread across ACT (exact exp + convert), DVE and Pool (tensor_scalar
affine -> uint8 -> bitcast fp8) to keep all three engines under the PE
roofline.

Schedule: query-chunk-outer / head-inner attention; out-proj + LN2 + the
xn2t transposes for the first token half are emitted inside the second
query-chunk's head loop so they fill engine gaps instead of forming a
serial phase.  All 512-wide fp32 PSUMs share one 6-deep rotation; the ctx
accumulators + ctx transposes share a 2-deep rotation (8 banks total).
"""

from contextlib import ExitStack

import ml_dtypes
import numpy as np

import concourse.bass as bass
import concourse.mybir as mybir
import concourse.tile as tile
from concourse import bacc
from concourse.masks import make_identity

F32 = mybir.dt.float32
BF16 = mybir.dt.bfloat16
FP8 = mybir.dt.float8e4
U8 = mybir.dt.uint8
AF = mybir.ActivationFunctionType
ALU = mybir.AluOpType
DR = mybir.MatmulPerfMode.DoubleRow

B_FULL = 4
S_FULL = 2048
D_FULL = 1024
H_FULL = 16
FF_FULL = 2048
HD = 64
EPS = 1e-5
N_CORES = 8

# softmax constants (scores scale 1/8, constant shift -1.5)
SM_SCALE = float(HD) ** -0.5
SM_SHIFT = -1.5
# fast-exp affine in e4m3 byte space: byte = s*K8 + B8
K8 = SM_SCALE * 8.0 * np.log2(np.e)
B8 = 7 * 8 + SM_SHIFT * 8.0 * np.log2(np.e)

# exp engine schedule, cycled per exp-instruction: A=ACT exact, D=DVE fast,
# P=Pool fast
EXP_PAT = "ADADADAD"

LAST_EXEC_NS = None
LAST_RESULTS = None
LAST_NC = None


def build_nc(S=S_FULL, T=S_FULL // 2, D=D_FULL, H=H_FULL, FF=FF_FULL,
             gelu_af=AF.Gelu, zero_bv=False, zero_b2=False):
    """Build the single-core (SPMD) Bass program.

    S: total tokens per batch (K/V length), T: own tokens (Q length),
    D: model dim, H: heads (H*64 == D), FF: hidden dim.
    """
    assert H * HD == D
    P = 128
    DT = D // P           # d-tiles (contraction tiles over D)
    TT_ALL = S // P       # token tiles over full sequence
    TT_OWN = T // P       # token tiles over own tokens
    FT = FF // P          # ff tiles
    QC = min(512, T)      # q chunk (columns per scores matmul)
    NQC = T // QC
    QSUB = QC // P
    NC_D = min(512, D)    # matmul N chunk over D
    DCH = D // NC_D
    NG = 2                # bn_stats groups
    GS = D // NG
    NHG = H // 4          # head groups of 4 (one [128,2,S] kt tile each)

    nc = bacc.Bacc("TRN2", target_bir_lowering=False, debug=False,
                   enable_asserts=False, num_devices=N_CORES)

    xpb_d = nc.dram_tensor("xpb", [T, D], F32, kind="ExternalInput").ap()
    xb_d = nc.dram_tensor("xb", [S, D], FP8, kind="ExternalInput").ap()
    wq_d = nc.dram_tensor("wq", [D, D], FP8, kind="ExternalInput").ap()
    wk_d = nc.dram_tensor("wk", [D, D], FP8, kind="ExternalInput").ap()
    wv_d = nc.dram_tensor("wv", [D, D], FP8, kind="ExternalInput").ap()
    wo_d = nc.dram_tensor("wo", [D, D], FP8, kind="ExternalInput").ap()
    w1_d = nc.dram_tensor("w1", [D, FF], FP8, kind="ExternalInput").ap()
    w2_d = nc.dram_tensor("w2", [FF, D], FP8, kind="ExternalInput").ap()
    bq_d = nc.dram_tensor("bq", [D], F32, kind="ExternalInput").ap()
    bk_d = nc.dram_tensor("bk", [D], F32, kind="ExternalInput").ap()
    bv_d = nc.dram_tensor("bv", [D], F32, kind="ExternalInput").ap()
    b1_d = nc.dram_tensor("b1", [FF], F32, kind="ExternalInput").ap()
    b2_d = nc.dram_tensor("b2", [D], F32, kind="ExternalInput").ap()
    out_d = nc.dram_tensor("out", [T, D], F32, kind="ExternalOutput").ap()

    def bcast(ap_1d, n):
        return bass.AP(tensor=ap_1d.tensor, offset=ap_1d.offset,
                       ap=[[0, n]] + list(ap_1d.ap))

    exp_idx = [0]
    cp_idx = [0]
    expt_bufs = 4 if (zero_bv and zero_b2) else 3

    with tile.TileContext(nc) as tc:
      with ExitStack() as stack:
        ps_pool = stack.enter_context(
            tc.tile_pool(name="ps", bufs=1, space="PSUM"))

        def psum(shape, dtype=F32):
            return ps_pool.tile(shape, dtype, tag="sc", name="pst", bufs=3)

        def psum_ctx(shape, dtype=F32):
            return ps_pool.tile(shape, dtype, tag="ps4", name="ps4", bufs=2)

        small = stack.enter_context(tc.tile_pool(name="small", bufs=1))
        ident = small.tile([P, P], BF16, name="ident")
        make_identity(nc, ident)
        eps_t = small.tile([P, 1], F32, name="eps_t")
        nc.vector.memset(eps_t, EPS)
        shift_t = small.tile([P, 1], F32, name="shift_t")
        nc.vector.memset(shift_t, SM_SHIFT)
        bq_sb = small.tile([P, DT], F32, name="bq_sb")
        nc.sync.dma_start(out=bq_sb, in_=bq_d.rearrange("(t p) -> p t", p=P))
        bk_sb = small.tile([P, DT], F32, name="bk_sb")
        nc.sync.dma_start(out=bk_sb, in_=bk_d.rearrange("(t p) -> p t", p=P))
        b1_sb = small.tile([P, FT], F32, name="b1_sb")
        nc.sync.dma_start(out=b1_sb, in_=b1_d.rearrange("(t p) -> p t", p=P))
        if not zero_bv:
            bv_bc = small.tile([P, D], F32, name="bv_bc")
            nc.gpsimd.dma_start(out=bv_bc, in_=bcast(bv_d, P))
        if not zero_b2:
            b2_bc = small.tile([P, D], F32, name="b2_bc")
            nc.gpsimd.dma_start(out=b2_bc, in_=bcast(b2_d, P))

        # ---- right-side stack bottom: tensors that survive into the FFN ----
        p_w1 = tc.alloc_tile_pool(name="p_w1", bufs=1, side="right")
        w1_sb = p_w1.tile([P, DT, FF], FP8, name="w1_sb")
        p_w2 = tc.alloc_tile_pool(name="p_w2", bufs=1, side="right")
        w2_sb = p_w2.tile([P, FT, D], FP8, name="w2_sb")
        p_ht = tc.alloc_tile_pool(name="p_ht", bufs=1, side="right")
        ht = p_ht.tile([P, FT, T], FP8, name="ht")        # hT [ff, tok]
        p_x2 = tc.alloc_tile_pool(name="p_x2", bufs=1, side="right")
        x2 = p_x2.tile([P, TT_OWN, D], F32, name="x2")
        p_xn2t = tc.alloc_tile_pool(name="p_xn2t", bufs=1, side="right")
        xn2t = p_xn2t.tile([P, DT, TT_OWN, P], FP8, name="xn2t")

        # ---- right-side stack: LN1/QKV phase (released innermost-first) ----
        p_xnt = tc.alloc_tile_pool(name="p_xnt", bufs=1, side="right")
        xnt = p_xnt.tile([P, DT, TT_ALL, P], FP8, name="xnt")
        p_wv = tc.alloc_tile_pool(name="p_wv", bufs=1, side="right")
        wv_sb = p_wv.tile([P, DT, D], FP8, name="wv_sb")
        p_wk = tc.alloc_tile_pool(name="p_wk", bufs=1, side="right")
        wk_sb = p_wk.tile([P, DT, D], FP8, name="wk_sb")
        p_wq = tc.alloc_tile_pool(name="p_wq", bufs=1, side="right")
        wq_sb = p_wq.tile([P, DT, D], FP8, name="wq_sb")
        p_xall = tc.alloc_tile_pool(name="p_xall", bufs=1, side="right")
        x_all = p_xall.tile([P, TT_ALL, D], FP8, name="x_all")
        # x_all first (LN1 is the first consumer), then Q/K/V weights
        for tt in range(TT_ALL):
            nc.sync.dma_start(out=x_all[:, tt, :],
                              in_=xb_d[P * tt:P * (tt + 1), :])
        for dt in range(DT):
            nc.sync.dma_start(out=wq_sb[:, dt, :],
                              in_=wq_d[P * dt:P * (dt + 1), :])
        for dt in range(DT):
            nc.sync.dma_start(out=wk_sb[:, dt, :],
                              in_=wk_d[P * dt:P * (dt + 1), :])
        for dt in range(DT):
            nc.sync.dma_start(out=wv_sb[:, dt, :],
                              in_=wv_d[P * dt:P * (dt + 1), :])
        for dt in range(DT):
            nc.sync.dma_start(out=w1_sb[:, dt, :],
                              in_=w1_d[P * dt:P * (dt + 1), :])
        for ft in range(FT):
            nc.sync.dma_start(out=w2_sb[:, ft, :],
                              in_=w2_d[P * ft:P * (ft + 1), :])

        def emit_ln(pool, x_sl, tt, xn_t):
            """LayerNorm stats + normalized write into bf16 xn_t.
            Alternates DVE (bn_stats) and ACT (accum) paths per token tile."""
            act_path = tt % 2 == 1
            mean = pool.tile([P, 1], F32, tag="mean", name="mean")
            var = pool.tile([P, 1], F32, tag="var", name="var")
            if not act_path:
                stats = pool.tile([P, NG, 6], F32, tag="st", name="stats")
                for g in range(NG):
                    nc.vector.bn_stats(out=stats[:, g, :],
                                       in_=x_sl[:, GS * g:GS * (g + 1)])
                mv = pool.tile([P, 2], F32, tag="mv", name="mv")
                nc.vector.bn_aggr(out=mv, in_=stats)
                nc.vector.tensor_copy(out=mean, in_=mv[:, 0:1])
                nc.vector.tensor_copy(out=var, in_=mv[:, 1:2])
            else:
                scr = pool.tile([P, D], BF16, tag="scr", name="scr")
                s1 = pool.tile([P, 1], F32, tag="s1", name="s1")
                ssq = pool.tile([P, 1], F32, tag="ssq", name="ssq")
                nc.scalar.activation(out=scr, in_=x_sl, func=AF.Identity,
                                     accum_out=s1)
                nc.scalar.activation(out=scr, in_=x_sl, func=AF.Square,
                                     accum_out=ssq)
                nc.vector.tensor_scalar(out=mean, in0=s1, scalar1=1.0 / D,
                                        scalar2=None, op0=ALU.mult)
                m2 = pool.tile([P, 1], F32, tag="m2", name="m2")
                nc.vector.tensor_tensor(out=m2, in0=mean, in1=mean,
                                        op=ALU.mult)
                nc.vector.tensor_scalar(out=var, in0=ssq, scalar1=1.0 / D,
                                        scalar2=None, op0=ALU.mult)
                nc.vector.tensor_tensor(out=var, in0=var, in1=m2,
                                        op=ALU.subtract)
            std = pool.tile([P, 1], F32, tag="sd", name="std")
            nc.scalar.activation(out=std, in_=var, func=AF.Sqrt,
                                 bias=eps_t, scale=1.0)
            rstd = pool.tile([P, 1], F32, tag="rs", name="rstd")
            nc.vector.reciprocal(out=rstd, in_=std)
            if tt % 2 == 0:
                nc.gpsimd.tensor_scalar(out=xn_t, in0=x_sl,
                                        scalar1=mean, scalar2=rstd,
                                        op0=ALU.subtract, op1=ALU.mult)
            else:
                nb = pool.tile([P, 1], F32, tag="nb", name="nb")
                nc.vector.tensor_scalar(out=nb, in0=mean, scalar1=rstd,
                                        scalar2=-1.0, op0=ALU.mult,
                                        op1=ALU.mult)
                nc.scalar.activation(out=xn_t, in_=x_sl, func=AF.Identity,
                                     bias=nb, scale=rstd)

        def emit_transposes(xn_t, dst, tt):
            """PE-transpose bf16 xn_t into fp8 dst[:, :, tt, :]; one batched
            psum -> one SBUF copy, engine round-robined."""
            tp = psum([P, DT * P], BF16)
            for j in range(DT):
                nc.tensor.transpose(
                    tp[:, P * j:P * (j + 1)],
                    xn_t[:, P * j:P * (j + 1)], ident)
            which = cp_idx[0] % 2
            cp_idx[0] += 1
            if which == 0:
                nc.vector.tensor_copy(out=dst[:, :, tt, :], in_=tp)
            else:
                nc.scalar.activation(out=dst[:, :, tt, :], in_=tp,
                                     func=AF.Identity)

        # ---------------- LN1 + transpose into xnt ----------------
        ln_pool = tc.alloc_tile_pool(name="ln_pool", bufs=5, side="right")
        for tt in range(TT_ALL):
            xn_t = ln_pool.tile([P, D], BF16, tag="xn", name="xn_t")
            emit_ln(ln_pool, x_all[:, tt, :], tt, xn_t)
            emit_transposes(xn_t, xnt, tt)
        ln_pool.release()
        p_xall.release()

        # ---- left-side stack: attention-lifetime tensors ----
        p_ctxt = tc.alloc_tile_pool(name="p_ctxt", bufs=1, side="left")
        ctxt = p_ctxt.tile([P, DT, T], FP8, name="ctxt")   # ctxT [d, tok]
        p_wo = tc.alloc_tile_pool(name="p_wo", bufs=1, side="left")
        wo_sb = p_wo.tile([P, DT, D], FP8, name="wo_sb")
        for dt in range(DT):
            nc.sync.dma_start(out=wo_sb[:, dt, :],
                              in_=wo_d[P * dt:P * (dt + 1), :])
        p_qt = tc.alloc_tile_pool(name="p_qt", bufs=1, side="left")
        # qT in scores layout: [32q.., g, half, tok]
        qt = p_qt.tile([P, NHG, 2, T], FP8, name="qt")

        # ------------- Q projection (transposed output) -------------
        # permuted block b holds (head-group b//2, dim-half b%2)
        QPC = min(1024, T)
        for b in range(DT):
            for c in range(T // QPC):
                ps = psum([P, QPC])
                for j in range(QPC // 512):
                    t0 = (QPC * c + 512 * j) // P
                    for dt in range(0, DT, 2):
                        nc.tensor.matmul(
                            ps[:, 512 * j:512 * (j + 1)],
                            wq_sb[:, dt:dt + 2, P * b:P * (b + 1)],
                            xnt[:, dt:dt + 2, t0:t0 + 4, :],
                            start=(dt == 0), stop=(dt == DT - 2),
                            perf_mode=DR)
                qdst = qt[:, b // 2, b % 2, QPC * c:QPC * (c + 1)]
                if b % 2 == 0:
                    nc.scalar.activation(out=qdst, in_=ps, func=AF.Identity,
                                         bias=bq_sb[:, b:b + 1])
                else:
                    nc.vector.tensor_scalar(out=qdst, in0=ps,
                                            scalar1=bq_sb[:, b:b + 1],
                                            scalar2=None, op0=ALU.add)
        p_wq.release()

        ln2_pool = tc.alloc_tile_pool(name="ln2_pool", bufs=3, side="right")
        p_kt = tc.alloc_tile_pool(name="p_kt", bufs=4, side="left")
        p_va = tc.alloc_tile_pool(name="p_va", bufs=1, side="left")
        v_aug = p_va.tile([P, TT_ALL, H, HD + 1], FP8, name="v_aug")
        nc.vector.memset(v_aug[:, :, :, HD:HD + 1], 1.0)

        def emit_kproj(g, kt_t=None, parts=None):
            """kT for head group g: [128, 2, S] (partitions 32q hold head
            4g+q; free dim 1 holds the two 32-dim halves).  `parts` selects a
            subset of (half, chunk) pieces so emission can be spread."""
            if kt_t is None:
                kt_t = p_kt.tile([P, 2, S], FP8, tag="ktt", name="kt_t")
            tkc = min(1024, S)
            tpc = tkc // P
            for half in range(2):
                b = 2 * g + half
                for c in range(S // tkc):
                    if parts is not None and (half, c) not in parts:
                        continue
                    ps = psum([P, tkc])
                    for j in range(tkc // 512):
                        t0 = tpc * c + 4 * j
                        for dt in range(0, DT, 2):
                            nc.tensor.matmul(
                                ps[:, 512 * j:512 * (j + 1)],
                                wk_sb[:, dt:dt + 2, P * b:P * (b + 1)],
                                xnt[:, dt:dt + 2, t0:t0 + 4, :],
                                start=(dt == 0), stop=(dt == DT - 2),
                                perf_mode=DR)
                    kdst = kt_t[:, half, tkc * c:tkc * (c + 1)]
                    if (half + c) % 2 == 0:
                        nc.vector.tensor_scalar(out=kdst, in0=ps,
                                                scalar1=bk_sb[:, b:b + 1],
                                                scalar2=None, op0=ALU.add)
                    else:
                        nc.scalar.activation(out=kdst, in_=ps,
                                             func=AF.Identity,
                                             bias=bk_sb[:, b:b + 1])
            return kt_t

        def emit_vproj(tts):
            for tt in tts:
                ps = psum([P, D])
                for j in range(D // 512):
                    for dt in range(0, DT, 2):
                        nc.tensor.matmul(
                            ps[:, 512 * j:512 * (j + 1)],
                            xnt[:, dt:dt + 2, tt, :],
                            wv_sb[:, dt:dt + 2, 512 * j:512 * (j + 1)],
                            start=(dt == 0), stop=(dt == DT - 2),
                            perf_mode=DR)
                dst = v_aug[:, tt, :, 0:HD]
                if not zero_bv:
                    nc.vector.tensor_tensor(out=dst, in0=ps, in1=bv_bc,
                                            op=ALU.add)
                elif tt % 2 == 0:
                    nc.scalar.activation(out=dst, in_=ps, func=AF.Identity)
                else:
                    nc.vector.tensor_copy(out=dst, in_=ps)


        def fetch_xpb(tt):
            # residual lands directly in x2; out-proj accumulates in place
            nc.sync.dma_start(out=x2[:, tt, :],
                              in_=xpb_d[P * tt:P * (tt + 1), :])

        def emit_outproj(tt):
            """out-proj + residual for token tile tt."""
            ps = psum([P, D])
            for j in range(D // 512):
                for dt in range(0, DT, 2):
                    nc.tensor.matmul(
                        ps[:, 512 * j:512 * (j + 1)],
                        ctxt[:, dt:dt + 2, P * tt:P * (tt + 1)],
                        wo_sb[:, dt:dt + 2, 512 * j:512 * (j + 1)],
                        start=(dt == 0), stop=(dt == DT - 2), perf_mode=DR)
            nc.vector.tensor_tensor(out=x2[:, tt, :], in0=ps,
                                    in1=x2[:, tt, :], op=ALU.add)

        def emit_ln2(tt):
            """LN2 + xn2t transposes for token tile tt."""
            xn_t = ln2_pool.tile([P, D], BF16, tag="xn", name="xn2_t")
            emit_ln(ln2_pool, x2[:, tt, :], tt, xn_t)
            emit_transposes(xn_t, xn2t, tt)

        def emit_outproj_ln2(tt):
            emit_outproj(tt)
            emit_ln2(tt)

        tkc = min(256, T)
        tpc = tkc // P

        def emit_fc1(c, ft0):
            if True:
                ps = psum([P, 2 * tkc])
                for j in range(2):
                    ft = ft0 + j
                    for dt in range(0, DT, 2):
                        nc.tensor.matmul(
                            ps[:, tkc * j:tkc * (j + 1)],
                            w1_sb[:, dt:dt + 2, P * ft:P * (ft + 1)],
                            xn2t[:, dt:dt + 2, tpc * c:tpc * (c + 1), :],
                            start=(dt == 0), stop=(dt == DT - 2),
                            perf_mode=DR)
                # per-partition bias differs between the two ft blocks only
                # via b1_sb columns; gelu is emitted per block to keep the
                # bias correct but reads the shared psum
                for j in range(2):
                    ft = ft0 + j
                    nc.scalar.activation(
                        out=ht[:, ft, tkc * c:tkc * (c + 1)],
                        in_=ps[:, tkc * j:tkc * (j + 1)],
                        func=gelu_af, bias=b1_sb[:, ft:ft + 1], scale=1.0)

        def emit_fc2(tt):
            ps = psum([P, D])
            for j in range(D // 512):
                for ft in range(0, FT, 2):
                    nc.tensor.matmul(
                        ps[:, 512 * j:512 * (j + 1)],
                        ht[:, ft:ft + 2, P * tt:P * (tt + 1)],
                        w2_sb[:, ft:ft + 2, 512 * j:512 * (j + 1)],
                        start=(ft == 0), stop=(ft == FT - 2), perf_mode=DR)
            # x2[:, tt, :] is dead after this add: accumulate the final
            # output in place and DMA straight from it
            nc.vector.tensor_tensor(out=x2[:, tt, :], in0=ps,
                                    in1=x2[:, tt, :], op=ALU.add)
            if not zero_b2:
                nc.vector.tensor_tensor(out=x2[:, tt, :], in0=x2[:, tt, :],
                                        in1=b2_bc, op=ALU.add)
            nc.sync.dma_start(out=out_d[P * tt:P * (tt + 1), :],
                              in_=x2[:, tt, :])

        # ---------------- attention ----------------
        # Query-chunk-outer / head-inner; software-pipelined so scores+exp of
        # chunk i are emitted before the ctx block of chunk i-1.  During the
        # second query chunk, out-proj/LN2 work for the first chunk's tokens
        # is woven between head iterations.
        exp_pool = tc.alloc_tile_pool(name="exp_pool", bufs=1, side="left")
        ctx_pool = tc.alloc_tile_pool(name="ctx_pool", bufs=3, side="left")

        HT = TT_ALL // 2

        def emit_exp(ps, dst):
            eng = EXP_PAT[exp_idx[0] % len(EXP_PAT)]
            exp_idx[0] += 1
            if eng == "A":
                nc.scalar.activation(out=dst, in_=ps, func=AF.Exp,
                                     scale=SM_SCALE, bias=shift_t)
            elif eng == "D":
                nc.vector.tensor_scalar(out=dst.bitcast(U8), in0=ps,
                                        scalar1=float(K8), scalar2=float(B8),
                                        op0=ALU.mult, op1=ALU.add)
            else:
                nc.gpsimd.tensor_scalar(out=dst.bitcast(U8), in0=ps,
                                        scalar1=float(K8), scalar2=float(B8),
                                        op0=ALU.mult, op1=ALU.add)

        def emit_scores(h, qc, kt_t):
            g, q = h // 4, h % 4
            po = 32 * q
            halves = []
            for hf in range(2):
                expt = exp_pool.tile([P, HT, QC], FP8, tag="expt",
                                     name="expt", bufs=expt_bufs)
                for j0 in range(0, HT, 2):
                    ps = psum([P, 2 * QC])
                    for jj in range(2):
                        st = hf * HT + j0 + jj
                        nc.tensor.matmul(
                            ps[:, QC * jj:QC * (jj + 1)],
                            kt_t[po:po + 32, :, P * st:P * (st + 1)],
                            qt[po:po + 32, g, :, QC * qc:QC * (qc + 1)],
                            start=True, stop=True, perf_mode=DR,
                            tile_position=(po, 0))
                    emit_exp(ps, expt[:, j0:j0 + 2, :])
                halves.append(expt)
            return halves

        def emit_ctx(h, qc, halves):
            po = HD * (h % 2)
            dot = h // 2
            ctp = ps_pool.tile([HD, QSUB * P], BF16, tag="ps4", name="ctp",
                               bufs=2)
            ps4 = psum_ctx([P, QSUB, HD + 1])
            for k in range(QSUB):
                for st0 in range(0, TT_ALL, 2):
                    expt = halves[st0 // HT]
                    nc.tensor.matmul(
                        ps4[:, k, :],
                        expt[:, st0 % HT:st0 % HT + 2, P * k:P * (k + 1)],
                        v_aug[:, st0:st0 + 2, h, :],
                        start=(st0 == 0), stop=(st0 == TT_ALL - 2),
                        perf_mode=DR)
            rec = ctx_pool.tile([P, QSUB], F32, tag="rec", name="rec")
            nc.vector.reciprocal(out=rec, in_=ps4[:, :, HD])
            for k in range(QSUB):
                csb = ctx_pool.tile([P, HD], BF16, tag="csb", name="csb",
                                    bufs=6)
                nc.scalar.activation(out=csb, in_=ps4[:, k, 0:HD],
                                     func=AF.Identity,
                                     scale=rec[:, k:k + 1])
                nc.tensor.transpose(ctp[:, P * k:P * (k + 1)], csb, ident)
            nc.vector.tensor_copy(
                out=ctxt[po:po + HD, dot, QC * qc:QC * (qc + 1)], in_=ctp)

        for tt in range(min(2, TT_OWN)):
            fetch_xpb(tt)

        kt_ts = [emit_kproj(0)]
        prev = None
        for qc in range(NQC):
            for h in range(H):
                if qc == 0:
                    g_next, piece = h // 4 + 1, h % 4
                    if g_next < NHG:
                        if piece == 0:
                            kt_ts.append(emit_kproj(
                                g_next, parts=[(0, 0)]))
                        else:
                            emit_kproj(g_next, kt_t=kt_ts[g_next],
                                       parts=[(piece // 2, piece % 2)])
                    if h == 0:
                        emit_vproj(range(0, TT_ALL // 2))
                    if h == 1:
                        emit_vproj(range(TT_ALL // 2, TT_ALL))
                elif h >= 2 and (h - 2) // 4 < QSUB:
                    # weave first-half out-proj/LN2 + the FFN pipeline for
                    # already-finished token pairs between head iterations,
                    # one small piece per head iteration
                    tt, piece = (h - 2) // 4, (h - 2) % 4
                    if piece == 0:
                        if tt + 2 < TT_OWN:
                            fetch_xpb(tt + 2)
                        emit_outproj(tt)
                    elif piece == 1:
                        emit_ln2(tt)
                    elif tt % 2 == 1:
                        ch = tt // 2      # token pair (2ch, 2ch+1) done
                        fh = range(0, FT // 2, 2) if piece == 2 else \
                            range(FT // 2, FT, 2)
                        for ft0 in fh:
                            emit_fc1(ch, ft0)
                    elif tt == 2:
                        emit_fc2(piece - 2)   # ht chunk 0 ready
                if prev is not None:
                    emit_ctx(*prev)
                prev = (h, qc, emit_scores(h, qc, kt_ts[h // 4]))
        emit_ctx(*prev)
        for tt in range(QSUB, TT_OWN):
            if tt + 2 < TT_OWN:
                fetch_xpb(tt + 2)
            emit_outproj_ln2(tt)
            if tt == QSUB:
                # token pair (2,3)'s fc1 didn't fit in the weave window
                for ft0 in range(0, FT, 2):
                    emit_fc1(1, ft0)
            if tt % 2 == 1:
                ch = tt // 2
                for ft0 in range(0, FT, 2):
                    emit_fc1(ch, ft0)
                emit_fc2(2 * ch - 2)
                emit_fc2(2 * ch - 1)
        emit_fc2(TT_OWN - 2)
        emit_fc2(TT_OWN - 1)
        ln2_pool.release()
        ctx_pool.release()
        exp_pool.release()
        p_va.release()
        p_kt.release()
        p_qt.release()
        p_wo.release()
        p_ctxt.release()
        p_wk.release()
        p_wv.release()
        p_xnt.release()

        # ---------------- FFN ----------------



        p_xn2t.release()
        p_x2.release()
        p_ht.release()
        p_w2.release()
        p_w1.release()
    nc.compile()
    return nc


def _qk_perm(D=D_FULL):
    """Column permutation for Wq/Wk: block b holds (head-group b//2,
    dim-half b%2); partitions 32q..32q+31 of a block hold head 4*(b//2)+q."""
    perm = np.empty(D, dtype=np.int64)
    for p_col in range(D):
        b, p = divmod(p_col, 128)
        g, half = divmod(b, 2)
        head = 4 * g + p // 32
        dim = 32 * half + p % 32
        perm[p_col] = 64 * head + dim
    return perm


def _fold_host(inputs):
    """Fold LN affine + biases into weights (fp32), permute Q/K columns for
    the DoubleRow scores layout, cast weights to fp8e4 (e4m3)."""
    f = {k: np.asarray(v, dtype=np.float32) for k, v in inputs.items()}
    g1, b1, g2, b2 = f["g1"], f["b1"], f["g2"], f["b2"]
    perm = _qk_perm(f["Wq"].shape[0])
    f8 = lambda a: np.ascontiguousarray(a).astype(ml_dtypes.float8_e4m3)
    w = {
        "wq": f8((g1[:, None] * f["Wq"])[:, perm]),
        "wk": f8((g1[:, None] * f["Wk"])[:, perm]),
        "wv": f8(g1[:, None] * f["Wv"]),
        "wo": f8(f["Wo"]),
        "w1": f8(g2[:, None] * f["W1"]),
        "w2": f8(f["W2"]),
        "bq": np.ascontiguousarray((b1 @ f["Wq"] + f["bq"])[perm]),
        "bk": np.ascontiguousarray((b1 @ f["Wk"] + f["bk"])[perm]),
        "bv": np.ascontiguousarray(f["bv"]),
        "b1": np.ascontiguousarray(b2 @ f["W1"] + f["bf1"]),
        "b2": np.ascontiguousarray(f["bf2"]),
    }
    return f, w


def kernel(**inputs):
    global LAST_EXEC_NS, LAST_RESULTS, LAST_NC
    import os

    from concourse.bass_utils import run_bass_kernel_spmd

    f, w = _fold_host(inputs)
    x = f["x"]
    B, S, D = x.shape
    T = S // 2
    zero_bv = not np.any(w["bv"])
    zero_b2 = not np.any(w["b2"])
    nc = build_nc(S=S, T=T, D=D, H=H_FULL, FF=FF_FULL,
                  zero_bv=zero_bv, zero_b2=zero_b2)
    LAST_NC = nc

    in_maps = []
    for c in range(N_CORES):
        b, half = c // 2, c % 2
        if half == 0:
            xb = x[b]
        else:
            xb = np.concatenate([x[b, T:], x[b, :T]], axis=0)
        m = {"xpb": np.ascontiguousarray(xb[:T] + f["bo"][None, :]),
             "xb": np.ascontiguousarray(xb).astype(ml_dtypes.float8_e4m3)}
        m.update(w)
        in_maps.append(m)

    trace = bool(int(os.environ.get("KBENCH_TRACE", "0")))
    res = run_bass_kernel_spmd(nc, in_maps, list(range(N_CORES)), trace=trace)
    LAST_EXEC_NS = res.exec_time_ns
    LAST_RESULTS = res

    out = np.empty((B, S, D), dtype=np.float32)
    for c in range(N_CORES):
        b, half = c // 2, c % 2
        out[b, T * half:T * (half + 1)] = res.results[c]["out"]
    return out
